# revision 21
# baseline (speedup 1.0000x reference)
"""Trainium2 Bass kernel for BracketGNN (3-layer GCN + mean-pool + MLP head).

Strategy (8 NeuronCores, SPMD):
  - Nodes sharded across cores (6250 dst nodes / core, padded blocks of 128);
    each core owns the incident in-edges of its dst nodes.
  - Node slice split into piece A (blocks 0-24, 3200 nodes) and piece B
    (blocks 25-48, 3050 nodes). Per layer the table T_l[n] = dinv_n*(h_l@W'_l)
    is exchanged with TWO AllGathers (T_A then T_B), launched as soon as the
    producing half of the blocks is done, so collectives overlap compute:
    layer l+1's A-flavor gathers need only T_A(l+1), which finishes during
    layer l's B phase.
  - Edge messages are fetched with the MoE dma_gather primitive (int16
    piece-local indices), round-robined over 4 SWDGE queues so descriptor
    generation runs on all four Q7 CPU pairs concurrently.
  - Aggregation = per-128-edge-chunk matmul: psum[feat, dst] +=
    msg[128e x 128f].T @ seg[128e x 128dst], seg built on-device by
    iota==dstcol compare. Layer processing is two-phase: phase A accumulates
    A-flavor chunks into an SBUF slab (pre-scaled by dinv_d, bf16), phase B
    accumulates B-flavor chunks in PSUM and the epilogue combines
    (aggB + T_l[d])*dinv_d + aggA  -- the T_l[d] term realizes the GCN
    self-loops, which are therefore dropped from the edge streams.
  - Epilogue stays feature-major: +bias(folded BN), ReLU, next-layer weight
    transform, x dinv_n into a stash slab (reused for the self-loop term),
    PE-transpose to node rows only for the AllGather input.
  - After layer 3: per-graph mean pool via one-hot graph matmul (graph
    matrix streamed from DRAM per block), AllReduce of the [128f x 256g]
    partial sums, then the replicated fp32 classifier head.
"""

import sys
import numpy as np

sys.path.insert(0, "/opt/trn_rl_repo")

import ml_dtypes

BF16 = ml_dtypes.bfloat16

# ---------------------------------------------------------------- real config
REAL_CFG = dict(
    N=50000, E=800000, IN=128, H=128, C=16, G=256, NC=8,
    EPS=1e-5, GCH=14, NBLK_A=25,
)


# ---------------------------------------------------------------- preprocess
def preprocess(inputs, cfg):
    """Host-side: shard + build all per-core data arrays and the static meta
    that shapes the program (identical across cores)."""
    N, NC, G, H = cfg["N"], cfg["NC"], cfg["G"], cfg["H"]
    IN = cfg["IN"]
    NPC = N // NC
    NBLK = (NPC + 127) // 128
    NBLK_A = cfg["NBLK_A"]
    NBLK_B = NBLK - NBLK_A
    NPC_A = NBLK_A * 128
    NPC_B = NPC - NPC_A
    LASTROWS = NPC - (NBLK - 1) * 128
    NA, NB = NC * NPC_A, NC * NPC_B
    assert NA - 1 <= 32767 and NB - 1 <= 32767, "int16 gather index overflow"

    ei = np.asarray(inputs["edge_index"]).astype(np.int64)
    batch = np.asarray(inputs["batch"]).astype(np.int64)
    x = np.asarray(inputs["x"], np.float32)
    scalar = np.asarray(inputs["scalar"], np.float32)

    # degrees INCLUDE the implicit self loops; edge streams EXCLUDE them
    # (the self-loop message T_l[d] is folded into the epilogue from the
    # locally stashed table rows).
    src_all = ei[0].copy()
    dst_all = ei[1].copy()
    deg = (np.bincount(np.concatenate([dst_all, np.arange(N)]), minlength=N)
           .astype(np.float32))
    dinv = 1.0 / np.sqrt(np.maximum(deg, 1.0))

    # ---- relabel nodes. Pass 1: split each core's nodes into piece A
    # (NPC_A) / piece B by snaking in-degree order, fixing every edge's
    # flavor = piece(src). Pass 2: within each piece, greedily pack nodes
    # into blocks to balance the per-block (A_in, B_in) loads that define
    # the SPMD-uniform chunk schedule.
    tot_in = np.bincount(dst_all, minlength=N).astype(np.int64)
    in_a = np.zeros(N, np.int64)
    piece_a = np.zeros(N, bool)
    for r in range(NC):
        nodes = np.arange(r * NPC, (r + 1) * NPC)
        order = nodes[np.argsort(-tot_in[nodes], kind="stable")]
        # ratio fill in in-degree order keeps A/B in-degree shares ~ capacities
        ca, cb = NPC_A, NPC_B
        for n in order:
            if ca > 0 and (cb == 0 or ca * NPC_B >= cb * NPC_A):
                piece_a[n] = True
                ca -= 1
            else:
                cb -= 1
    src_is_a = piece_a[src_all]
    in_a = np.bincount(dst_all[src_is_a], minlength=N).astype(np.int64)
    in_b = tot_in - in_a

    gp = np.empty(N, np.int64)  # old global id -> new global id
    for r in range(NC):
        nodes = np.arange(r * NPC, (r + 1) * NPC)
        for pa in (True, False):
            sub = nodes[piece_a[nodes] == pa]
            nblk = NBLK_A if pa else NBLK_B
            base = 0 if pa else NPC_A
            cap = np.full(nblk, 128, np.int64)
            if not pa:
                cap[nblk - 1] = LASTROWS
            order = np.argsort(-(in_a[sub] + in_b[sub]), kind="stable")
            a_tgt = max(float(in_a[sub].sum()) / nblk, 1.0)
            b_tgt = max(float(in_b[sub].sum()) / nblk, 1.0)
            a_load = np.zeros(nblk, np.float64)
            b_load = np.zeros(nblk, np.float64)
            fill = np.zeros(nblk, np.int64)
            for o in order:
                n = sub[o]
                cand = np.nonzero(fill < cap)[0]
                score = np.maximum((a_load[cand] + in_a[n]) / a_tgt,
                                   (b_load[cand] + in_b[n]) / b_tgt)
                bsel = cand[np.argmin(score)]
                gp[n] = r * NPC + base + bsel * 128 + fill[bsel]
                a_load[bsel] += in_a[n]
                b_load[bsel] += in_b[n]
                fill[bsel] += 1

    src_all = gp[src_all]
    dst_all = gp[dst_all]
    x2 = np.empty_like(x)
    x2[gp] = x
    x = x2
    batch2 = np.empty_like(batch)
    batch2[gp] = batch
    batch = batch2
    dinv2 = np.empty_like(dinv)
    dinv2[gp] = dinv
    dinv = dinv2

    # piece-local source index (flavor A/B by NEW local id)
    s_core = src_all // NPC
    s_loc = src_all % NPC
    s_is_a = s_loc < NPC_A
    s_pid = np.where(s_is_a, s_core * NPC_A + s_loc,
                     s_core * NPC_B + (s_loc - NPC_A))

    # BN folding: layer l: h' = relu((agg + b - m) * k * g + be)
    gamma = np.asarray(inputs["gamma"], np.float32)
    beta = np.asarray(inputs["beta"], np.float32)
    mean = np.asarray(inputs["mean"], np.float32)
    var = np.asarray(inputs["var"], np.float32)
    W0 = np.asarray(inputs["W0"], np.float32)
    Wk = np.asarray(inputs["Wk"], np.float32)
    b0 = np.asarray(inputs["b0"], np.float32)
    bk = np.asarray(inputs["bk"], np.float32)

    Wl = [W0, Wk[0], Wk[1]]
    bl = [b0, bk[0], bk[1]]
    A = [gamma[i] / np.sqrt(var[i] + cfg["EPS"]) for i in range(3)]
    B = [(bl[i] - mean[i]) * A[i] + beta[i] for i in range(3)]
    Wp = [Wl[i] * A[i][None, :] for i in range(3)]  # W'_l

    # ---- per-core edge lists, chunked per (block, flavor) -----------------
    per_core = []
    cpb_a_max = 0
    cpb_b_max = 0
    for r in range(NC):
        m = (dst_all >= r * NPC) & (dst_all < (r + 1) * NPC)
        s = s_pid[m]
        fa = s_is_a[m]
        d = dst_all[m] - r * NPC
        blk = d // 128
        blocks = []
        for b in range(NBLK):
            mb = blk == b
            sb = s[mb]
            fb = fa[mb]
            db = (d[mb] - b * 128).astype(np.int64)
            pairs = []
            for flavor in (0, 1):
                mf = fb if flavor == 0 else ~fb
                # merge duplicate (src, dst) pairs into one slot with weight
                key = sb[mf] * 128 + db[mf]
                uk, wt = np.unique(key, return_counts=True)
                sf = uk // 128
                df = uk % 128
                nch = int(np.ceil(len(sf) / 128)) if len(sf) else 0
                pairs.append((sf.astype(np.int64), df, nch,
                              wt.astype(np.float32)))
            blocks.append(pairs)
            cpb_a_max = max(cpb_a_max, pairs[0][2])
            cpb_b_max = max(cpb_b_max, pairs[1][2])
        per_core.append(blocks)

    GCH = cfg["GCH"]
    CPB = (cpb_a_max, cpb_b_max)
    NLO = CPB[0] * NBLK
    NHI = CPB[1] * NBLK
    NCALL_LO = -(-NLO // GCH) if NLO else 0
    NCALL_HI = -(-NHI // GCH) if NHI else 0
    NLO_P = NCALL_LO * GCH
    NCH = NLO_P + NCALL_HI * GCH

    # ---- build per-core tensors -------------------------------------------
    in_maps = []
    for r in range(NC):
        idx_flat = np.zeros((NCH, 128), np.int16)
        segval = np.zeros((NCH, 128), np.float32)
        segdst = np.zeros((NCH, 128), np.int64)
        for b in range(NBLK):
            for flavor in (0, 1):
                sf, df, nch, wt = per_core[r][b][flavor]
                base = (b * CPB[0]) if flavor == 0 else (NLO_P + b * CPB[1])
                for j in range(nch):
                    lo_e = j * 128
                    hi_e = min(lo_e + 128, len(sf))
                    n = hi_e - lo_e
                    c = base + j
                    idx_flat[c, :n] = sf[lo_e:hi_e].astype(np.int16)
                    segval[c, :n] = wt[lo_e:hi_e]
                    segdst[c, :n] = df[lo_e:hi_e]
        # host-built weighted one-hot segs [128(edge), NCH, 128(dst-local)]
        seg_host = np.zeros((128, NCH, 128), BF16)
        cc, pp = np.nonzero(segval > 0)
        seg_host[pp, cc, segdst[cc, pp]] = segval[cc, pp].astype(BF16)
        idx16 = idx_flat.reshape(NCH, 8, 16).transpose(2, 0, 1).reshape(16, NCH * 8)
        idx128 = np.tile(idx16, (8, 1))

        xs = x[r * NPC:(r + 1) * NPC]
        xT = np.zeros((IN, NBLK * 128), np.float32)
        xT[:, :NPC] = xs.T
        dinv_r = np.zeros((NBLK * 128,), np.float32)
        dinv_r[:NPC] = dinv[r * NPC:(r + 1) * NPC]
        dinvrep = np.broadcast_to(dinv_r, (128, NBLK * 128)).astype(BF16)

        # graph one-hot [128, NBLK, G] (partition-major for clean DMA)
        gm = np.zeros((128, NBLK, G), np.float32)
        bt = batch[r * NPC:(r + 1) * NPC]
        nn = np.arange(NPC)
        gm[nn % 128, nn // 128, bt] = 1.0

        counts = np.bincount(batch, minlength=G).astype(np.float32)
        cinv = 1.0 / np.maximum(counts, 1.0)
        cinvrep = np.broadcast_to(cinv, (128, G)).copy()

        in_map = dict(
            xT=xT.astype(BF16),
            seg=seg_host,
            idx=idx128.astype(np.int16),
            dinvrep=dinvrep,
            Bcols=np.stack(B, axis=1).astype(np.float32),          # [H, 3]
            W0p=Wp[0].astype(BF16),
            W1p=Wp[1].astype(BF16),
            W2p=Wp[2].astype(BF16),
            ident=np.eye(128, dtype=BF16),
            gm=gm.astype(BF16),
            cinvrep=cinvrep,
            scalarT=scalar.T.copy().astype(np.float32),            # [8, G]
            Ws=np.asarray(inputs["Ws"], np.float32),
            bs_col=np.asarray(inputs["bs"], np.float32).reshape(-1, 1),
            Wc1a=np.asarray(inputs["Wc1"], np.float32)[:H].copy(),
            Wc1b=np.asarray(inputs["Wc1"], np.float32)[H:].copy(),
            bc1_col=np.asarray(inputs["bc1"], np.float32).reshape(-1, 1),
            Wc2=np.asarray(inputs["Wc2"], np.float32),
            bc2_col=np.asarray(inputs["bc2"], np.float32).reshape(-1, 1),
        )
        in_maps.append(in_map)

    meta = dict(
        cfg=cfg, NPC=NPC, NBLK=NBLK, NBLK_A=NBLK_A, LASTROWS=LASTROWS,
        NPC_A=NPC_A, NPC_B=NPC_B, NA=NA, NB=NB,
        CPB=CPB, NLO=NLO, NHI=NHI, NCH=NCH, NLO_P=NLO_P,
        NCALL_LO=NCALL_LO, NCALL_HI=NCALL_HI,
        shapes={k: (tuple(v.shape), v.dtype) for k, v in in_maps[0].items()},
    )
    return meta, in_maps


# ---------------------------------------------------------------- program
def build_program(meta):
    import concourse.bacc as bacc
    import concourse.mybir as mybir
    import concourse.tile as tile

    cfg = meta["cfg"]
    NC, G, H, IN = cfg["NC"], cfg["G"], cfg["H"], cfg["IN"]
    C, GCH = cfg["C"], cfg["GCH"]
    NBLK, NBLK_A, LASTROWS = meta["NBLK"], meta["NBLK_A"], meta["LASTROWS"]
    NPC_A, NPC_B, NA, NB = meta["NPC_A"], meta["NPC_B"], meta["NA"], meta["NB"]
    CPB, NCH, NLO_P = meta["CPB"], meta["NCH"], meta["NLO_P"]
    NCALL_LO, NCALL_HI = meta["NCALL_LO"], meta["NCALL_HI"]
    f32, bf16, i16 = mybir.dt.float32, mybir.dt.bfloat16, mybir.dt.int16
    Alu = mybir.AluOpType
    Act = mybir.ActivationFunctionType

    nc = bacc.Bacc("TRN2", target_bir_lowering=False, debug=False,
                   enable_asserts=True, num_devices=NC,
                   num_swdge_queues=4)

    def dram_in(name):
        shape, dtype = meta["shapes"][name]
        return nc.dram_tensor(name, list(shape), mybir.dt.from_np(np.dtype(dtype)),
                              kind="ExternalInput").ap()

    ins = {k: dram_in(k) for k in meta["shapes"]}
    out_dram = nc.dram_tensor("out", [C, G], f32, kind="ExternalOutput").ap()

    rg = [list(range(NC))]

    # gather call plan: stream = [A calls][B calls], each exactly GCH chunks
    calls = [(0, ci * GCH) for ci in range(NCALL_LO)] + \
            [(1, NLO_P + ci * GCH) for ci in range(NCALL_HI)]
    call_of_chunk = {}
    for ci, (fl, start) in enumerate(calls):
        for k in range(GCH):
            call_of_chunk[start + k] = (ci, k)

    with tile.TileContext(nc) as tc:
        with (
            tc.tile_pool(name="const", bufs=1) as constp,
            tc.tile_pool(name="msgs", bufs=9) as msgp,
            tc.tile_pool(name="segs", bufs=9) as segp,
            tc.tile_pool(name="work", bufs=3) as workp,
            tc.tile_pool(name="rows", bufs=3) as rowp,
            tc.tile_pool(name="psum", bufs=2, space="PSUM") as psump,
            tc.tile_pool(name="psumT", bufs=2, space="PSUM") as psumTp,
            tc.tile_pool(name="psumU", bufs=2, space="PSUM") as psumUp,
            tc.tile_pool(name="psumP", bufs=1, space="PSUM") as psumPp,
            tc.tile_pool(name="dram", bufs=1, space="DRAM") as dramp,
        ):
            # ---------------- constants into SBUF
            idx_sb = constp.tile([128, NCH * 8], i16)
            nc.sync.dma_start(idx_sb[:], ins["idx"][:])
            seg_dram = ins["seg"]
            dinvrep_sb = constp.tile([128, NBLK * 128], bf16)
            nc.sync.dma_start(dinvrep_sb[:], ins["dinvrep"][:])
            Bcols_sb = constp.tile([H, 3], f32)
            nc.sync.dma_start(Bcols_sb[:], ins["Bcols"][:])
            W_sb = []
            for wname in ("W0p", "W1p", "W2p"):
                w = constp.tile([H, H], bf16, name=f"w_{wname}")
                nc.sync.dma_start(w[:], ins[wname][:])
                W_sb.append(w)
            ident_sb = constp.tile([128, 128], bf16)
            nc.sync.dma_start(ident_sb[:], ins["ident"][:])
            cinvrep_sb = constp.tile([128, G], f32)
            nc.sync.dma_start(cinvrep_sb[:], ins["cinvrep"][:])
            scalarT_sb = constp.tile([8, G], f32)
            nc.sync.dma_start(scalarT_sb[:], ins["scalarT"][:])
            Ws_sb = constp.tile([8, 64], f32)
            nc.sync.dma_start(Ws_sb[:], ins["Ws"][:])
            bs_sb = constp.tile([64, 1], f32)
            nc.sync.dma_start(bs_sb[:], ins["bs_col"][:])
            Wc1a_sb = constp.tile([H, H], f32)
            nc.sync.dma_start(Wc1a_sb[:], ins["Wc1a"][:])
            Wc1b_sb = constp.tile([64, H], f32)
            nc.sync.dma_start(Wc1b_sb[:], ins["Wc1b"][:])
            bc1_sb = constp.tile([H, 1], f32)
            nc.sync.dma_start(bc1_sb[:], ins["bc1_col"][:])
            Wc2_sb = constp.tile([H, C], f32)
            nc.sync.dma_start(Wc2_sb[:], ins["Wc2"][:])
            bc2_sb = constp.tile([C, 1], f32)
            nc.sync.dma_start(bc2_sb[:], ins["bc2_col"][:])
            gm_dram = ins["gm"]

            # stash of current-layer table rows (feature-major, x dinv) and
            # the phase-A partial aggregate slab (pre-scaled by dinv_d)
            stash_sb = constp.tile([H, NBLK * 128], bf16, name="stash")
            aggA_sb = constp.tile([H, NBLK * 128], bf16, name="aggA")

            # ---------------- DRAM tables + collective buffers
            shared = "Shared" if NC > 4 else "Local"
            TA = [dramp.tile([NA, H], bf16, name=f"tableA{l}", addr_space=shared)
                  for l in range(3)]
            TB = [dramp.tile([NB, H], bf16, name=f"tableB{l}", addr_space=shared)
                  for l in range(3)]
            aginA = [dramp.tile([NPC_A, H], bf16, name=f"aginA{l}")
                     for l in range(3)]
            aginB = [dramp.tile([NPC_B, H], bf16, name=f"aginB{l}")
                     for l in range(3)]
            ar_in = dramp.tile([128, G], f32, name="ar_in")
            ar_out = dramp.tile([128, G], f32, name="ar_out", addr_space=shared)

            def launch_ag(l, piece):
                src = aginA[l] if piece == 0 else aginB[l]
                dst = TA[l] if piece == 0 else TB[l]
                nc.gpsimd.collective_compute(
                    "AllGather", Alu.bypass, replica_groups=rg,
                    ins=[src.opt()], outs=[dst.opt()],
                )

            def write_rows(l, b, src_sb):
                """PE-transpose feature-major block src_sb [H,128] into node
                rows and DMA them into the agin buffer for layer l."""
                tps = psumTp.tile([128, H], bf16, tag="tpsum")
                nc.tensor.transpose(tps[:], src_sb, ident_sb[:])
                rows = rowp.tile([128, H], bf16, tag="rows")
                nc.scalar.activation(rows[:], tps[:], Act.Copy)
                if b < NBLK_A:
                    lo = b * 128
                    nc.sync.dma_start(aginA[l][lo:lo + 128, :], rows[:])
                else:
                    lo = (b - NBLK_A) * 128
                    nr = 128 if b < NBLK - 1 else LASTROWS
                    nc.sync.dma_start(aginB[l][lo:lo + nr, :], rows[:nr, :])

            # ---------------- stage A: T0 = dinv * (x @ W0')
            for b in range(NBLK):
                xb = rowp.tile([IN, 128], bf16, tag="xb", name=f"xb_{b}")
                nc.sync.dma_start(xb[:], ins["xT"][:, b * 128:(b + 1) * 128])
                u0 = psumUp.tile([H, 128], f32, tag="upsum")
                nc.tensor.matmul(u0[:], W_sb[0][:], xb[:],
                                 start=True, stop=True)
                st = stash_sb[:, b * 128:(b + 1) * 128]
                nc.vector.tensor_tensor(
                    st, u0[:], dinvrep_sb[:, b * 128:(b + 1) * 128],
                    op=Alu.mult)
                write_rows(0, b, st)
                if b == NBLK_A - 1:
                    launch_ag(0, 0)
            launch_ag(0, 1)

            # ---------------- layers
            for l in range(3):
                msg_tiles = {}
                seg_tiles = {}

                def ensure_call(ci, l=l, msg_tiles=msg_tiles,
                                seg_tiles=seg_tiles):
                    if ci in msg_tiles:
                        return
                    fl, start = calls[ci]
                    mt = msgp.tile([128, GCH, H], bf16, tag=f"msg{fl}",
                                   name=f"msg_{l}_{ci}")
                    st = segp.tile([128, GCH, 128], bf16, tag=f"seg{fl}",
                                   name=f"seg_{l}_{ci}")
                    view = TA[l][:] if fl == 0 else TB[l][:]
                    nidx = GCH * 128
                    nc.gpsimd.dma_gather(
                        mt[:], view, idx_sb[:, start * 8:(start + GCH) * 8],
                        nidx, nidx, H, single_packet=False,
                        queue_num=ci % 4,
                    )
                    nc.scalar.dma_start(st[:], seg_dram[:, start:start + GCH, :])
                    msg_tiles[ci] = mt
                    seg_tiles[ci] = st

                # ---- phase A: A-flavor partial aggregates -> SBUF slab
                for b in range(NBLK):
                    agg = psump.tile([H, 128], f32, tag="agg",
                                     name=f"aggA_{l}_{b}")
                    for k in range(CPB[0]):
                        ci, slot = call_of_chunk[b * CPB[0] + k]
                        ensure_call(ci)
                        nc.tensor.matmul(
                            agg[:], msg_tiles[ci][:, slot, :],
                            seg_tiles[ci][:, slot, :],
                            start=(k == 0), stop=False, skip_group_check=True,
                        )
                    nc.vector.tensor_tensor(
                        aggA_sb[:, b * 128:(b + 1) * 128], agg[:],
                        dinvrep_sb[:, b * 128:(b + 1) * 128], op=Alu.mult)

                # ---- phase B + epilogue
                for b in range(NBLK):
                    agg = psump.tile([H, 128], f32, tag="agg",
                                     name=f"aggB_{l}_{b}")
                    for k in range(CPB[1]):
                        ci, slot = call_of_chunk[NLO_P + b * CPB[1] + k]
                        ensure_call(ci)
                        nc.tensor.matmul(
                            agg[:], msg_tiles[ci][:, slot, :],
                            seg_tiles[ci][:, slot, :],
                            start=(k == 0), stop=False, skip_group_check=True,
                        )
                    sl = slice(b * 128, (b + 1) * 128)
                    # (aggB + T_l[d]) * dinv_d + aggA_prescaled
                    t0 = workp.tile([H, 128], f32, tag="t0")
                    nc.vector.tensor_tensor(t0[:], agg[:], stash_sb[:, sl],
                                            op=Alu.add)
                    t1 = workp.tile([H, 128], f32, tag="t1")
                    nc.vector.tensor_tensor(t1[:], t0[:], dinvrep_sb[:, sl],
                                            op=Alu.mult)
                    tmp = workp.tile([H, 128], f32, tag="tmp")
                    nc.vector.tensor_tensor(tmp[:], t1[:], aggA_sb[:, sl],
                                            op=Alu.add)
                    hT = workp.tile([H, 128], bf16, tag="hT")
                    nc.scalar.activation(hT[:], tmp[:], Act.Relu,
                                         bias=Bcols_sb[:, l:l + 1])
                    if l < 2:
                        ups = psumUp.tile([H, 128], f32, tag="upsum")
                        nc.tensor.matmul(ups[:], W_sb[l + 1][:], hT[:],
                                         start=True, stop=True)
                        st = stash_sb[:, sl]
                        nc.vector.tensor_tensor(st, ups[:],
                                                dinvrep_sb[:, sl],
                                                op=Alu.mult)
                        write_rows(l + 1, b, st)
                        if b == NBLK_A - 1:
                            launch_ag(l + 1, 0)
                        elif b == NBLK - 1:
                            launch_ag(l + 1, 1)
                    else:
                        tps = psumTp.tile([128, H], bf16, tag="tpsum")
                        nc.tensor.transpose(tps[:], hT[:], ident_sb[:])
                        h3 = workp.tile([128, H], bf16, tag="h3")
                        nc.vector.tensor_copy(h3[:], tps[:])
                        gmb = rowp.tile([128, G], bf16, tag="gmb",
                                        name=f"gmb_{b}")
                        nc.sync.dma_start(gmb[:], gm_dram[:, b, :])
                        if b == 0:
                            pool_ps = psumPp.tile([H, G], f32, name="pool_ps")
                        nc.tensor.matmul(pool_ps[:], h3[:], gmb[:],
                                         start=(b == 0), stop=(b == NBLK - 1),
                                         skip_group_check=True)

            # ---------------- pooled partial -> AllReduce
            pool_sb = workp.tile([H, G], f32, name="pool_sb")
            nc.vector.tensor_copy(pool_sb[:], pool_ps[:])
            nc.sync.dma_start(ar_in[:], pool_sb[:])
            nc.gpsimd.collective_compute(
                "AllReduce", Alu.add, replica_groups=rg,
                ins=[ar_in.opt()], outs=[ar_out.opt()],
            )
            gsum = workp.tile([H, G], f32, name="gsum")
            nc.sync.dma_start(gsum[:], ar_out[:])
            gembT = workp.tile([H, G], f32, name="gembT")
            nc.vector.tensor_tensor(gembT[:], gsum[:], cinvrep_sb[:], op=Alu.mult)

            # ---------------- head (fp32, replicated on every core)
            semb_ps = psumUp.tile([64, G], f32, tag="upsum", name="semb_ps")
            nc.tensor.matmul(semb_ps[:], Ws_sb[:], scalarT_sb[:],
                             start=True, stop=True)
            sembT = workp.tile([64, G], f32, name="sembT")
            nc.scalar.activation(sembT[:], semb_ps[:], Act.Relu, bias=bs_sb[:])

            z_ps = psumPp.tile([H, G], f32, name="z_ps")
            nc.tensor.matmul(z_ps[:], Wc1a_sb[:], gembT[:],
                             start=True, stop=False, skip_group_check=True)
            nc.tensor.matmul(z_ps[:], Wc1b_sb[:], sembT[:],
                             start=False, stop=True, skip_group_check=True)
            z2T = workp.tile([H, G], f32, name="z2T")
            nc.scalar.activation(z2T[:], z_ps[:], Act.Relu, bias=bc1_sb[:])

            o_ps = psumUp.tile([C, G], f32, tag="upsum", name="o_ps")
            nc.tensor.matmul(o_ps[:], Wc2_sb[:], z2T[:], start=True, stop=True)
            o_sb = workp.tile([C, G], f32, name="o_sb")
            nc.scalar.activation(o_sb[:], o_ps[:], Act.Identity, bias=bc2_sb[:])
            nc.sync.dma_start(out_dram[:], o_sb[:])

    nc.compile()
    return nc


# ---------------------------------------------------------------- runner
_CACHE = {}


def run(inputs, cfg=None, trace=False):
    cfg = cfg or REAL_CFG
    meta, in_maps = preprocess(inputs, cfg)
    key = (tuple(sorted(cfg.items())), meta["CPB"], meta["NCH"])
    if key not in _CACHE:
        _CACHE[key] = build_program(meta)
    nc = _CACHE[key]
    from concourse import bass_utils
    res = bass_utils.run_bass_kernel_spmd(
        nc, in_maps, core_ids=list(range(cfg["NC"])), trace=trace,
    )
    out = np.asarray(res.results[0]["out"], np.float32).T.copy()
    return out, res


def kernel(**inputs) -> np.ndarray:
    out, _ = run(inputs, REAL_CFG, trace=False)
    return out


# revision 34
# speedup vs baseline: 1.1898x; 1.1898x over previous
"""Trainium2 Bass kernel for BracketGNN (3-layer GCN + mean-pool + MLP head).

Strategy (8 NeuronCores, SPMD):
  - Nodes sharded across cores (6250 dst nodes / core, padded blocks of 128);
    each core owns the incident in-edges of its dst nodes.
  - Node slice split into piece A (blocks 0-24, 3200 nodes) and piece B
    (blocks 25-48, 3050 nodes). Per layer the table T_l[n] = dinv_n*(h_l@W'_l)
    is exchanged with TWO AllGathers (T_A then T_B), launched as soon as the
    producing half of the blocks is done, so collectives overlap compute:
    layer l+1's A-flavor gathers need only T_A(l+1), which finishes during
    layer l's B phase.
  - Edge messages are fetched with the MoE dma_gather primitive (int16
    piece-local indices), round-robined over 4 SWDGE queues so descriptor
    generation runs on all four Q7 CPU pairs concurrently.
  - Aggregation = per-128-edge-chunk matmul: psum[feat, dst] +=
    msg[128e x 128f].T @ seg[128e x 128dst], seg built on-device by
    iota==dstcol compare. Layer processing is two-phase: phase A accumulates
    A-flavor chunks into an SBUF slab (pre-scaled by dinv_d, bf16), phase B
    accumulates B-flavor chunks in PSUM and the epilogue combines
    (aggB + T_l[d])*dinv_d + aggA  -- the T_l[d] term realizes the GCN
    self-loops, which are therefore dropped from the edge streams.
  - Epilogue stays feature-major: +bias(folded BN), ReLU, next-layer weight
    transform, x dinv_n into a stash slab (reused for the self-loop term),
    PE-transpose to node rows only for the AllGather input.
  - After layer 3: per-graph mean pool via one-hot graph matmul (graph
    matrix streamed from DRAM per block), AllReduce of the [128f x 256g]
    partial sums, then the replicated fp32 classifier head.
"""

import sys
import numpy as np

sys.path.insert(0, "/opt/trn_rl_repo")

import ml_dtypes

BF16 = ml_dtypes.bfloat16

# ---------------------------------------------------------------- real config
REAL_CFG = dict(
    N=50000, E=800000, IN=128, H=128, C=16, G=256, NC=8,
    EPS=1e-5, GCH=14, NBLK_A=17,
)


# ---------------------------------------------------------------- preprocess
def preprocess(inputs, cfg):
    """Host-side: shard + build all per-core data arrays and the static meta
    that shapes the program (identical across cores)."""
    N, NC, G, H = cfg["N"], cfg["NC"], cfg["G"], cfg["H"]
    IN = cfg["IN"]
    NPC = N // NC
    NBLK = (NPC + 127) // 128
    NBLK_A = cfg["NBLK_A"]
    NBLK_B = NBLK - NBLK_A
    NPC_A = NBLK_A * 128
    NPC_B = NPC - NPC_A
    LASTROWS = NPC - (NBLK - 1) * 128
    NA, NB = NC * NPC_A, NC * NPC_B
    assert NA - 1 <= 32767 and NB - 1 <= 32767, "int16 gather index overflow"

    ei = np.asarray(inputs["edge_index"]).astype(np.int64)
    batch = np.asarray(inputs["batch"]).astype(np.int64)
    x = np.asarray(inputs["x"], np.float32)
    scalar = np.asarray(inputs["scalar"], np.float32)

    # degrees INCLUDE the implicit self loops; edge streams EXCLUDE them
    # (the self-loop message T_l[d] is folded into the epilogue from the
    # locally stashed table rows).
    src_all = ei[0].copy()
    dst_all = ei[1].copy()
    deg = (np.bincount(np.concatenate([dst_all, np.arange(N)]), minlength=N)
           .astype(np.float32))
    dinv = 1.0 / np.sqrt(np.maximum(deg, 1.0))

    # ---- relabel nodes. Pass 1: split each core's nodes into piece A
    # (NPC_A) / piece B by snaking in-degree order, fixing every edge's
    # flavor = piece(src). Pass 2: within each piece, greedily pack nodes
    # into blocks to balance the per-block (A_in, B_in) loads that define
    # the SPMD-uniform chunk schedule.
    tot_in = np.bincount(dst_all, minlength=N).astype(np.int64)
    in_a = np.zeros(N, np.int64)
    piece_a = np.zeros(N, bool)
    for r in range(NC):
        nodes = np.arange(r * NPC, (r + 1) * NPC)
        order = nodes[np.argsort(-tot_in[nodes], kind="stable")]
        # ratio fill in in-degree order keeps A/B in-degree shares ~ capacities
        ca, cb = NPC_A, NPC_B
        for n in order:
            if ca > 0 and (cb == 0 or ca * NPC_B >= cb * NPC_A):
                piece_a[n] = True
                ca -= 1
            else:
                cb -= 1
    src_is_a = piece_a[src_all]
    in_a = np.bincount(dst_all[src_is_a], minlength=N).astype(np.int64)
    in_b = tot_in - in_a

    gp = np.empty(N, np.int64)  # old global id -> new global id
    for r in range(NC):
        nodes = np.arange(r * NPC, (r + 1) * NPC)
        for pa in (True, False):
            sub = nodes[piece_a[nodes] == pa]
            nblk = NBLK_A if pa else NBLK_B
            base = 0 if pa else NPC_A
            cap = np.full(nblk, 128, np.int64)
            if not pa:
                cap[nblk - 1] = LASTROWS
            order = np.argsort(-(in_a[sub] + in_b[sub]), kind="stable")
            a_tgt = max(float(in_a[sub].sum()) / nblk, 1.0)
            b_tgt = max(float(in_b[sub].sum()) / nblk, 1.0)
            a_load = np.zeros(nblk, np.float64)
            b_load = np.zeros(nblk, np.float64)
            fill = np.zeros(nblk, np.int64)
            for o in order:
                n = sub[o]
                cand = np.nonzero(fill < cap)[0]
                score = np.maximum((a_load[cand] + in_a[n]) / a_tgt,
                                   (b_load[cand] + in_b[n]) / b_tgt)
                bsel = cand[np.argmin(score)]
                gp[n] = r * NPC + base + bsel * 128 + fill[bsel]
                a_load[bsel] += in_a[n]
                b_load[bsel] += in_b[n]
                fill[bsel] += 1

    src_all = gp[src_all]
    dst_all = gp[dst_all]
    x2 = np.empty_like(x)
    x2[gp] = x
    x = x2
    batch2 = np.empty_like(batch)
    batch2[gp] = batch
    batch = batch2
    dinv2 = np.empty_like(dinv)
    dinv2[gp] = dinv
    dinv = dinv2

    # piece-local source index (flavor A/B by NEW local id)
    s_core = src_all // NPC
    s_loc = src_all % NPC
    s_is_a = s_loc < NPC_A
    s_pid = np.where(s_is_a, s_core * NPC_A + s_loc,
                     s_core * NPC_B + (s_loc - NPC_A))

    # BN folding: layer l: h' = relu((agg + b - m) * k * g + be)
    gamma = np.asarray(inputs["gamma"], np.float32)
    beta = np.asarray(inputs["beta"], np.float32)
    mean = np.asarray(inputs["mean"], np.float32)
    var = np.asarray(inputs["var"], np.float32)
    W0 = np.asarray(inputs["W0"], np.float32)
    Wk = np.asarray(inputs["Wk"], np.float32)
    b0 = np.asarray(inputs["b0"], np.float32)
    bk = np.asarray(inputs["bk"], np.float32)

    Wl = [W0, Wk[0], Wk[1]]
    bl = [b0, bk[0], bk[1]]
    A = [gamma[i] / np.sqrt(var[i] + cfg["EPS"]) for i in range(3)]
    B = [(bl[i] - mean[i]) * A[i] + beta[i] for i in range(3)]
    Wp = [Wl[i] * A[i][None, :] for i in range(3)]  # W'_l

    # ---- per-core edge lists, chunked per (block, flavor) -----------------
    per_core = []
    cpb_a_max = 0
    cpb_b_max = 0
    for r in range(NC):
        m = (dst_all >= r * NPC) & (dst_all < (r + 1) * NPC)
        s = s_pid[m]
        fa = s_is_a[m]
        d = dst_all[m] - r * NPC
        blk = d // 128
        blocks = []
        for b in range(NBLK):
            mb = blk == b
            sb = s[mb]
            fb = fa[mb]
            db = (d[mb] - b * 128).astype(np.int64)
            pairs = []
            for flavor in (0, 1):
                mf = fb if flavor == 0 else ~fb
                sf = sb[mf]
                df = db[mf]
                nch = int(np.ceil(len(sf) / 128)) if len(sf) else 0
                pairs.append((sf.astype(np.int64), df, nch))
            blocks.append(pairs)
            cpb_a_max = max(cpb_a_max, pairs[0][2])
            cpb_b_max = max(cpb_b_max, pairs[1][2])
        per_core.append(blocks)

    GCH = cfg["GCH"]
    CPB = (cpb_a_max, cpb_b_max)
    NLO = CPB[0] * NBLK
    NHI = CPB[1] * NBLK
    NCALL_LO = -(-NLO // GCH) if NLO else 0
    NCALL_HI = -(-NHI // GCH) if NHI else 0
    NLO_P = NCALL_LO * GCH
    NCH = NLO_P + NCALL_HI * GCH

    # ---- build per-core tensors -------------------------------------------
    in_maps = []
    for r in range(NC):
        idx_flat = np.zeros((NCH, 128), np.int16)
        segval = np.zeros((NCH, 128), np.float32)
        segdst = np.zeros((NCH, 128), np.int64)
        for b in range(NBLK):
            for flavor in (0, 1):
                sf, df, nch = per_core[r][b][flavor]
                base = (b * CPB[0]) if flavor == 0 else (NLO_P + b * CPB[1])
                for j in range(nch):
                    lo_e = j * 128
                    hi_e = min(lo_e + 128, len(sf))
                    n = hi_e - lo_e
                    c = base + j
                    idx_flat[c, :n] = sf[lo_e:hi_e].astype(np.int16)
                    segval[c, :n] = 1.0
                    segdst[c, :n] = df[lo_e:hi_e]
        # dst-local per edge, bf16 (dummy edges -> 200, matching no column of
        # the on-device iota compare). [128(edge partition), NCH]
        dstcol = np.where(segval.T > 0, segdst.T.astype(np.float32),
                          200.0).astype(BF16)
        idx16 = idx_flat.reshape(NCH, 8, 16).transpose(2, 0, 1).reshape(16, NCH * 8)
        idx128 = np.tile(idx16, (8, 1))

        xs = x[r * NPC:(r + 1) * NPC]
        xT = np.zeros((IN, NBLK * 128), np.float32)
        xT[:, :NPC] = xs.T
        dinv_r = np.zeros((NBLK * 128,), np.float32)
        dinv_r[:NPC] = dinv[r * NPC:(r + 1) * NPC]
        dinvrep = np.broadcast_to(dinv_r, (128, NBLK * 128)).astype(BF16)

        # graph one-hot [128, NBLK, G] (partition-major for clean DMA)
        gm = np.zeros((128, NBLK, G), np.float32)
        bt = batch[r * NPC:(r + 1) * NPC]
        nn = np.arange(NPC)
        gm[nn % 128, nn // 128, bt] = 1.0

        counts = np.bincount(batch, minlength=G).astype(np.float32)
        cinv = 1.0 / np.maximum(counts, 1.0)
        cinvrep = np.broadcast_to(cinv, (128, G)).copy()

        iota_big = np.broadcast_to(
            np.arange(128, dtype=np.float32),
            (128, GCH, 128)).astype(BF16).copy()
        in_map = dict(
            xT=xT.astype(BF16),
            dstcol=dstcol,
            iota=iota_big,
            idx=idx128.astype(np.int16),
            dinvrep=dinvrep,
            Bcols=np.stack(B, axis=1).astype(np.float32),          # [H, 3]
            W0p=Wp[0].astype(BF16),
            W1p=Wp[1].astype(BF16),
            W2p=Wp[2].astype(BF16),
            ident=np.eye(128, dtype=BF16),
            gm=gm.astype(BF16),
            cinvrep=cinvrep,
            scalarT=scalar.T.copy().astype(np.float32),            # [8, G]
            Ws=np.asarray(inputs["Ws"], np.float32),
            bs_col=np.asarray(inputs["bs"], np.float32).reshape(-1, 1),
            Wc1a=np.asarray(inputs["Wc1"], np.float32)[:H].copy(),
            Wc1b=np.asarray(inputs["Wc1"], np.float32)[H:].copy(),
            bc1_col=np.asarray(inputs["bc1"], np.float32).reshape(-1, 1),
            Wc2=np.asarray(inputs["Wc2"], np.float32),
            bc2_col=np.asarray(inputs["bc2"], np.float32).reshape(-1, 1),
        )
        in_maps.append(in_map)

    meta = dict(
        cfg=cfg, NPC=NPC, NBLK=NBLK, NBLK_A=NBLK_A, LASTROWS=LASTROWS,
        NPC_A=NPC_A, NPC_B=NPC_B, NA=NA, NB=NB,
        CPB=CPB, NLO=NLO, NHI=NHI, NCH=NCH, NLO_P=NLO_P,
        NCALL_LO=NCALL_LO, NCALL_HI=NCALL_HI,
        shapes={k: (tuple(v.shape), v.dtype) for k, v in in_maps[0].items()},
    )
    return meta, in_maps


# ---------------------------------------------------------------- program
def build_program(meta):
    import concourse.bacc as bacc
    import concourse.mybir as mybir
    import concourse.tile as tile

    cfg = meta["cfg"]
    NC, G, H, IN = cfg["NC"], cfg["G"], cfg["H"], cfg["IN"]
    C, GCH = cfg["C"], cfg["GCH"]
    NBLK, NBLK_A, LASTROWS = meta["NBLK"], meta["NBLK_A"], meta["LASTROWS"]
    NPC_A, NPC_B, NA, NB = meta["NPC_A"], meta["NPC_B"], meta["NA"], meta["NB"]
    CPB, NCH, NLO_P = meta["CPB"], meta["NCH"], meta["NLO_P"]
    NCALL_LO, NCALL_HI = meta["NCALL_LO"], meta["NCALL_HI"]
    f32, bf16, i16 = mybir.dt.float32, mybir.dt.bfloat16, mybir.dt.int16
    Alu = mybir.AluOpType
    Act = mybir.ActivationFunctionType

    nc = bacc.Bacc("TRN2", target_bir_lowering=False, debug=False,
                   enable_asserts=True, num_devices=NC,
                   num_swdge_queues=4)

    def dram_in(name):
        shape, dtype = meta["shapes"][name]
        return nc.dram_tensor(name, list(shape), mybir.dt.from_np(np.dtype(dtype)),
                              kind="ExternalInput").ap()

    ins = {k: dram_in(k) for k in meta["shapes"]}
    out_dram = nc.dram_tensor("out", [C, G], f32, kind="ExternalOutput").ap()

    rg = [list(range(NC))]

    # gather call plan: stream = [A calls][B calls], each exactly GCH chunks
    calls = [(0, ci * GCH) for ci in range(NCALL_LO)] + \
            [(1, NLO_P + ci * GCH) for ci in range(NCALL_HI)]
    call_of_chunk = {}
    for ci, (fl, start) in enumerate(calls):
        for k in range(GCH):
            call_of_chunk[start + k] = (ci, k)

    with tile.TileContext(nc) as tc:
        with (
            tc.tile_pool(name="const", bufs=1) as constp,
            tc.tile_pool(name="msgs", bufs=8) as msgp,
            tc.tile_pool(name="segs", bufs=8) as segp,
            tc.tile_pool(name="work", bufs=3) as workp,
            tc.tile_pool(name="rows", bufs=3) as rowp,
            tc.tile_pool(name="psum", bufs=2, space="PSUM") as psump,
            tc.tile_pool(name="psumT", bufs=2, space="PSUM") as psumTp,
            tc.tile_pool(name="psumU", bufs=2, space="PSUM") as psumUp,
            tc.tile_pool(name="psumP", bufs=1, space="PSUM") as psumPp,
            tc.tile_pool(name="dram", bufs=1, space="DRAM") as dramp,
        ):
            # ---------------- constants into SBUF
            idx_sb = constp.tile([128, NCH * 8], i16)
            nc.sync.dma_start(idx_sb[:], ins["idx"][:])
            dstcol_sb = constp.tile([128, NCH], bf16)
            nc.sync.dma_start(dstcol_sb[:], ins["dstcol"][:])
            iota_sb = constp.tile([128, GCH, 128], bf16)
            nc.sync.dma_start(iota_sb[:], ins["iota"][:])
            dinvrep_sb = constp.tile([128, NBLK * 128], bf16)
            nc.sync.dma_start(dinvrep_sb[:], ins["dinvrep"][:])
            Bcols_sb = constp.tile([H, 3], f32)
            nc.sync.dma_start(Bcols_sb[:], ins["Bcols"][:])
            W_sb = []
            for wname in ("W0p", "W1p", "W2p"):
                w = constp.tile([H, H], bf16, name=f"w_{wname}")
                nc.sync.dma_start(w[:], ins[wname][:])
                W_sb.append(w)
            ident_sb = constp.tile([128, 128], bf16)
            nc.sync.dma_start(ident_sb[:], ins["ident"][:])
            cinvrep_sb = constp.tile([128, G], f32)
            nc.sync.dma_start(cinvrep_sb[:], ins["cinvrep"][:])
            scalarT_sb = constp.tile([8, G], f32)
            nc.sync.dma_start(scalarT_sb[:], ins["scalarT"][:])
            Ws_sb = constp.tile([8, 64], f32)
            nc.sync.dma_start(Ws_sb[:], ins["Ws"][:])
            bs_sb = constp.tile([64, 1], f32)
            nc.sync.dma_start(bs_sb[:], ins["bs_col"][:])
            Wc1a_sb = constp.tile([H, H], f32)
            nc.sync.dma_start(Wc1a_sb[:], ins["Wc1a"][:])
            Wc1b_sb = constp.tile([64, H], f32)
            nc.sync.dma_start(Wc1b_sb[:], ins["Wc1b"][:])
            bc1_sb = constp.tile([H, 1], f32)
            nc.sync.dma_start(bc1_sb[:], ins["bc1_col"][:])
            Wc2_sb = constp.tile([H, C], f32)
            nc.sync.dma_start(Wc2_sb[:], ins["Wc2"][:])
            bc2_sb = constp.tile([C, 1], f32)
            nc.sync.dma_start(bc2_sb[:], ins["bc2_col"][:])
            gm_dram = ins["gm"]

            # stash of current-layer table rows (feature-major, x dinv) and
            # the phase-A partial aggregate slab (pre-scaled by dinv_d)
            stash_sb = constp.tile([H, NBLK * 128], bf16, name="stash")
            aggA_sb = constp.tile([H, NBLK * 128], bf16, name="aggA")

            # ---------------- DRAM tables + collective buffers
            shared = "Shared" if NC > 4 else "Local"
            TA = [dramp.tile([NA, H], bf16, name=f"tableA{l}", addr_space=shared)
                  for l in range(3)]
            TB = [dramp.tile([NB, H], bf16, name=f"tableB{l}", addr_space=shared)
                  for l in range(3)]
            aginA = [dramp.tile([NPC_A, H], bf16, name=f"aginA{l}")
                     for l in range(3)]
            aginB = [dramp.tile([NPC_B, H], bf16, name=f"aginB{l}")
                     for l in range(3)]
            ar_in = dramp.tile([128, G], f32, name="ar_in")
            ar_out = dramp.tile([128, G], f32, name="ar_out", addr_space=shared)

            def launch_ag(l, piece):
                src = aginA[l] if piece == 0 else aginB[l]
                dst = TA[l] if piece == 0 else TB[l]
                nc.gpsimd.collective_compute(
                    "AllGather", Alu.bypass, replica_groups=rg,
                    ins=[src.opt()], outs=[dst.opt()],
                )

            def write_rows(l, b, src_sb):
                """PE-transpose feature-major block src_sb [H,128] into node
                rows and DMA them into the agin buffer for layer l."""
                tps = psumTp.tile([128, H], bf16, tag="tpsum")
                nc.tensor.transpose(tps[:], src_sb, ident_sb[:])
                rows = rowp.tile([128, H], bf16, tag="rows")
                nc.scalar.activation(rows[:], tps[:], Act.Copy)
                if b < NBLK_A:
                    lo = b * 128
                    nc.sync.dma_start(aginA[l][lo:lo + 128, :], rows[:])
                else:
                    lo = (b - NBLK_A) * 128
                    nr = 128 if b < NBLK - 1 else LASTROWS
                    nc.sync.dma_start(aginB[l][lo:lo + nr, :], rows[:nr, :])

            # ---------------- stage A: T0 = dinv * (x @ W0')
            for b in range(NBLK):
                xb = rowp.tile([IN, 128], bf16, tag="xb", name=f"xb_{b}")
                nc.sync.dma_start(xb[:], ins["xT"][:, b * 128:(b + 1) * 128])
                u0 = psumUp.tile([H, 128], f32, tag="upsum")
                nc.tensor.matmul(u0[:], W_sb[0][:], xb[:],
                                 start=True, stop=True)
                st = stash_sb[:, b * 128:(b + 1) * 128]
                nc.vector.tensor_tensor(
                    st, u0[:], dinvrep_sb[:, b * 128:(b + 1) * 128],
                    op=Alu.mult)
                write_rows(0, b, st)
            launch_ag(0, 0)

            # ---------------- layers
            # AG triggers are emitted at points in the pool-engine stream
            # where their agin-write waits are already satisfied, so they
            # never head-of-line-block the gather queue:
            #   AG_B(l): after the 2nd A-call of layer l (input written by the
            #     end of layer l-1; the collective hides under the A phase).
            #   AG_A(l+1): once the B phase of layer l has ensured enough
            #     calls that blocks 0..NBLK_A-1 epilogues are done.
            aga_after = -(-NBLK_A * max(CPB[1], 1) // GCH) + 4
            for l in range(3):
                msg_tiles = {}
                seg_tiles = {}

                def ensure_call(ci, l=l, msg_tiles=msg_tiles,
                                seg_tiles=seg_tiles):
                    if ci in msg_tiles:
                        return
                    n_a = sum(1 for k in msg_tiles if calls[k][0] == 0)
                    n_b = len(msg_tiles) - n_a
                    if calls[ci][0] == 0 and n_a == 2:
                        launch_ag(l, 1)
                    if calls[ci][0] == 1 and n_b == aga_after and l < 2:
                        launch_ag(l + 1, 0)
                    fl, start = calls[ci]
                    mt = msgp.tile([128, GCH, H], bf16, tag=f"msg{fl}",
                                   name=f"msg_{l}_{ci}")
                    st = segp.tile([128, GCH, 128], bf16, tag=f"seg{fl}",
                                   name=f"seg_{l}_{ci}")
                    view = TA[l][:] if fl == 0 else TB[l][:]
                    nidx = GCH * 128
                    nc.gpsimd.dma_gather(
                        mt[:], view, idx_sb[:, start * 8:(start + GCH) * 8],
                        nidx, nidx, H, single_packet=False,
                        queue_num=ci % 4,
                    )
                    nc.vector.tensor_tensor(
                        st[:], iota_sb[:],
                        dstcol_sb[:, start:start + GCH][:, :, None]
                        .broadcast_to((128, GCH, 128)),
                        op=Alu.is_equal)
                    msg_tiles[ci] = mt
                    seg_tiles[ci] = st

                # ---- phase A: A-flavor partial aggregates -> SBUF slab
                for b in range(NBLK):
                    agg = psump.tile([H, 128], f32, tag="agg",
                                     name=f"aggA_{l}_{b}")
                    for k in range(CPB[0]):
                        ci, slot = call_of_chunk[b * CPB[0] + k]
                        ensure_call(ci)
                        nc.tensor.matmul(
                            agg[:], msg_tiles[ci][:, slot, :],
                            seg_tiles[ci][:, slot, :],
                            start=(k == 0), stop=False, skip_group_check=True,
                        )
                    nc.vector.tensor_tensor(
                        aggA_sb[:, b * 128:(b + 1) * 128], agg[:],
                        dinvrep_sb[:, b * 128:(b + 1) * 128], op=Alu.mult)

                # ---- phase B + epilogue
                for b in range(NBLK):
                    agg = psump.tile([H, 128], f32, tag="agg",
                                     name=f"aggB_{l}_{b}")
                    for k in range(CPB[1]):
                        ci, slot = call_of_chunk[NLO_P + b * CPB[1] + k]
                        ensure_call(ci)
                        nc.tensor.matmul(
                            agg[:], msg_tiles[ci][:, slot, :],
                            seg_tiles[ci][:, slot, :],
                            start=(k == 0), stop=False, skip_group_check=True,
                        )
                    sl = slice(b * 128, (b + 1) * 128)
                    # (aggB + T_l[d]) * dinv_d + aggA_prescaled
                    t0 = workp.tile([H, 128], f32, tag="t0")
                    nc.vector.tensor_tensor(t0[:], agg[:], stash_sb[:, sl],
                                            op=Alu.add)
                    t1 = workp.tile([H, 128], f32, tag="t1")
                    nc.vector.tensor_tensor(t1[:], t0[:], dinvrep_sb[:, sl],
                                            op=Alu.mult)
                    tmp = workp.tile([H, 128], f32, tag="tmp")
                    nc.vector.tensor_tensor(tmp[:], t1[:], aggA_sb[:, sl],
                                            op=Alu.add)
                    hT = workp.tile([H, 128], bf16, tag="hT")
                    nc.scalar.activation(hT[:], tmp[:], Act.Relu,
                                         bias=Bcols_sb[:, l:l + 1])
                    if l < 2:
                        ups = psumUp.tile([H, 128], f32, tag="upsum")
                        nc.tensor.matmul(ups[:], W_sb[l + 1][:], hT[:],
                                         start=True, stop=True)
                        st = stash_sb[:, sl]
                        nc.vector.tensor_tensor(st, ups[:],
                                                dinvrep_sb[:, sl],
                                                op=Alu.mult)
                        write_rows(l + 1, b, st)
                    else:
                        tps = psumTp.tile([128, H], bf16, tag="tpsum")
                        nc.tensor.transpose(tps[:], hT[:], ident_sb[:])
                        h3 = workp.tile([128, H], bf16, tag="h3")
                        nc.vector.tensor_copy(h3[:], tps[:])
                        gmb = rowp.tile([128, G], bf16, tag="gmb",
                                        name=f"gmb_{b}")
                        nc.sync.dma_start(gmb[:], gm_dram[:, b, :])
                        if b == 0:
                            pool_ps = psumPp.tile([H, G], f32, name="pool_ps")
                        nc.tensor.matmul(pool_ps[:], h3[:], gmb[:],
                                         start=(b == 0), stop=(b == NBLK - 1),
                                         skip_group_check=True)

            # ---------------- pooled partial -> AllReduce
            pool_sb = workp.tile([H, G], f32, name="pool_sb")
            nc.vector.tensor_copy(pool_sb[:], pool_ps[:])
            nc.sync.dma_start(ar_in[:], pool_sb[:])
            nc.gpsimd.collective_compute(
                "AllReduce", Alu.add, replica_groups=rg,
                ins=[ar_in.opt()], outs=[ar_out.opt()],
            )
            gsum = workp.tile([H, G], f32, name="gsum")
            nc.sync.dma_start(gsum[:], ar_out[:])
            gembT = workp.tile([H, G], f32, name="gembT")
            nc.vector.tensor_tensor(gembT[:], gsum[:], cinvrep_sb[:], op=Alu.mult)

            # ---------------- head (fp32, replicated on every core)
            semb_ps = psumUp.tile([64, G], f32, tag="upsum", name="semb_ps")
            nc.tensor.matmul(semb_ps[:], Ws_sb[:], scalarT_sb[:],
                             start=True, stop=True)
            sembT = workp.tile([64, G], f32, name="sembT")
            nc.scalar.activation(sembT[:], semb_ps[:], Act.Relu, bias=bs_sb[:])

            z_ps = psumPp.tile([H, G], f32, name="z_ps")
            nc.tensor.matmul(z_ps[:], Wc1a_sb[:], gembT[:],
                             start=True, stop=False, skip_group_check=True)
            nc.tensor.matmul(z_ps[:], Wc1b_sb[:], sembT[:],
                             start=False, stop=True, skip_group_check=True)
            z2T = workp.tile([H, G], f32, name="z2T")
            nc.scalar.activation(z2T[:], z_ps[:], Act.Relu, bias=bc1_sb[:])

            o_ps = psumUp.tile([C, G], f32, tag="upsum", name="o_ps")
            nc.tensor.matmul(o_ps[:], Wc2_sb[:], z2T[:], start=True, stop=True)
            o_sb = workp.tile([C, G], f32, name="o_sb")
            nc.scalar.activation(o_sb[:], o_ps[:], Act.Identity, bias=bc2_sb[:])
            nc.sync.dma_start(out_dram[:], o_sb[:])

    nc.compile()
    return nc


# ---------------------------------------------------------------- runner
_CACHE = {}


def run(inputs, cfg=None, trace=False):
    cfg = cfg or REAL_CFG
    meta, in_maps = preprocess(inputs, cfg)
    key = (tuple(sorted(cfg.items())), meta["CPB"], meta["NCH"])
    if key not in _CACHE:
        _CACHE[key] = build_program(meta)
    nc = _CACHE[key]
    from concourse import bass_utils
    res = bass_utils.run_bass_kernel_spmd(
        nc, in_maps, core_ids=list(range(cfg["NC"])), trace=trace,
    )
    out = np.asarray(res.results[0]["out"], np.float32).T.copy()
    return out, res


def kernel(**inputs) -> np.ndarray:
    out, _ = run(inputs, REAL_CFG, trace=False)
    return out


# revision 39
# speedup vs baseline: 1.1977x; 1.0066x over previous
"""Trainium2 Bass kernel for BracketGNN (3-layer GCN + mean-pool + MLP head).

Strategy (8 NeuronCores, SPMD):
  - Nodes sharded across cores (6250 dst nodes / core, padded blocks of 128);
    each core owns the incident in-edges of its dst nodes.
  - Node slice split into piece A (blocks 0-24, 3200 nodes) and piece B
    (blocks 25-48, 3050 nodes). Per layer the table T_l[n] = dinv_n*(h_l@W'_l)
    is exchanged with TWO AllGathers (T_A then T_B), launched as soon as the
    producing half of the blocks is done, so collectives overlap compute:
    layer l+1's A-flavor gathers need only T_A(l+1), which finishes during
    layer l's B phase.
  - Edge messages are fetched with the MoE dma_gather primitive (int16
    piece-local indices), round-robined over 4 SWDGE queues so descriptor
    generation runs on all four Q7 CPU pairs concurrently.
  - Aggregation = per-128-edge-chunk matmul: psum[feat, dst] +=
    msg[128e x 128f].T @ seg[128e x 128dst], seg built on-device by
    iota==dstcol compare. Layer processing is two-phase: phase A accumulates
    A-flavor chunks into an SBUF slab (pre-scaled by dinv_d, bf16), phase B
    accumulates B-flavor chunks in PSUM and the epilogue combines
    (aggB + T_l[d])*dinv_d + aggA  -- the T_l[d] term realizes the GCN
    self-loops, which are therefore dropped from the edge streams.
  - Epilogue stays feature-major: +bias(folded BN), ReLU, next-layer weight
    transform, x dinv_n into a stash slab (reused for the self-loop term),
    PE-transpose to node rows only for the AllGather input.
  - After layer 3: per-graph mean pool via one-hot graph matmul (graph
    matrix streamed from DRAM per block), AllReduce of the [128f x 256g]
    partial sums, then the replicated fp32 classifier head.
"""

import sys
import numpy as np

sys.path.insert(0, "/opt/trn_rl_repo")

import ml_dtypes

BF16 = ml_dtypes.bfloat16

# ---------------------------------------------------------------- real config
REAL_CFG = dict(
    N=50000, E=800000, IN=128, H=128, C=16, G=256, NC=8,
    EPS=1e-5, GCH=10, NBLK_A=17,
)


# ---------------------------------------------------------------- preprocess
def preprocess(inputs, cfg):
    """Host-side: shard + build all per-core data arrays and the static meta
    that shapes the program (identical across cores)."""
    N, NC, G, H = cfg["N"], cfg["NC"], cfg["G"], cfg["H"]
    IN = cfg["IN"]
    NPC = N // NC
    NBLK = (NPC + 127) // 128
    NBLK_A = cfg["NBLK_A"]
    NBLK_B = NBLK - NBLK_A
    NPC_A = NBLK_A * 128
    NPC_B = NPC - NPC_A
    LASTROWS = NPC - (NBLK - 1) * 128
    NA, NB = NC * NPC_A, NC * NPC_B
    assert NA - 1 <= 32767 and NB - 1 <= 32767, "int16 gather index overflow"

    ei = np.asarray(inputs["edge_index"]).astype(np.int64)
    batch = np.asarray(inputs["batch"]).astype(np.int64)
    x = np.asarray(inputs["x"], np.float32)
    scalar = np.asarray(inputs["scalar"], np.float32)

    # degrees INCLUDE the implicit self loops; edge streams EXCLUDE them
    # (the self-loop message T_l[d] is folded into the epilogue from the
    # locally stashed table rows).
    src_all = ei[0].copy()
    dst_all = ei[1].copy()
    deg = (np.bincount(np.concatenate([dst_all, np.arange(N)]), minlength=N)
           .astype(np.float32))
    dinv = 1.0 / np.sqrt(np.maximum(deg, 1.0))

    # ---- relabel nodes. Pass 1: split each core's nodes into piece A
    # (NPC_A) / piece B by snaking in-degree order, fixing every edge's
    # flavor = piece(src). Pass 2: within each piece, greedily pack nodes
    # into blocks to balance the per-block (A_in, B_in) loads that define
    # the SPMD-uniform chunk schedule.
    tot_in = np.bincount(dst_all, minlength=N).astype(np.int64)
    in_a = np.zeros(N, np.int64)
    piece_a = np.zeros(N, bool)
    for r in range(NC):
        nodes = np.arange(r * NPC, (r + 1) * NPC)
        order = nodes[np.argsort(-tot_in[nodes], kind="stable")]
        # ratio fill in in-degree order keeps A/B in-degree shares ~ capacities
        ca, cb = NPC_A, NPC_B
        for n in order:
            if ca > 0 and (cb == 0 or ca * NPC_B >= cb * NPC_A):
                piece_a[n] = True
                ca -= 1
            else:
                cb -= 1
    src_is_a = piece_a[src_all]
    in_a = np.bincount(dst_all[src_is_a], minlength=N).astype(np.int64)
    in_b = tot_in - in_a

    gp = np.empty(N, np.int64)  # old global id -> new global id
    for r in range(NC):
        nodes = np.arange(r * NPC, (r + 1) * NPC)
        for pa in (True, False):
            sub = nodes[piece_a[nodes] == pa]
            nblk = NBLK_A if pa else NBLK_B
            base = 0 if pa else NPC_A
            cap = np.full(nblk, 128, np.int64)
            if not pa:
                cap[nblk - 1] = LASTROWS
            order = np.argsort(-(in_a[sub] + in_b[sub]), kind="stable")
            a_tgt = max(float(in_a[sub].sum()) / nblk, 1.0)
            b_tgt = max(float(in_b[sub].sum()) / nblk, 1.0)
            a_load = np.zeros(nblk, np.float64)
            b_load = np.zeros(nblk, np.float64)
            fill = np.zeros(nblk, np.int64)
            for o in order:
                n = sub[o]
                cand = np.nonzero(fill < cap)[0]
                score = np.maximum((a_load[cand] + in_a[n]) / a_tgt,
                                   (b_load[cand] + in_b[n]) / b_tgt)
                bsel = cand[np.argmin(score)]
                gp[n] = r * NPC + base + bsel * 128 + fill[bsel]
                a_load[bsel] += in_a[n]
                b_load[bsel] += in_b[n]
                fill[bsel] += 1

    src_all = gp[src_all]
    dst_all = gp[dst_all]
    x2 = np.empty_like(x)
    x2[gp] = x
    x = x2
    batch2 = np.empty_like(batch)
    batch2[gp] = batch
    batch = batch2
    dinv2 = np.empty_like(dinv)
    dinv2[gp] = dinv
    dinv = dinv2

    # piece-local source index (flavor A/B by NEW local id)
    s_core = src_all // NPC
    s_loc = src_all % NPC
    s_is_a = s_loc < NPC_A
    s_pid = np.where(s_is_a, s_core * NPC_A + s_loc,
                     s_core * NPC_B + (s_loc - NPC_A))

    # BN folding: layer l: h' = relu((agg + b - m) * k * g + be)
    gamma = np.asarray(inputs["gamma"], np.float32)
    beta = np.asarray(inputs["beta"], np.float32)
    mean = np.asarray(inputs["mean"], np.float32)
    var = np.asarray(inputs["var"], np.float32)
    W0 = np.asarray(inputs["W0"], np.float32)
    Wk = np.asarray(inputs["Wk"], np.float32)
    b0 = np.asarray(inputs["b0"], np.float32)
    bk = np.asarray(inputs["bk"], np.float32)

    Wl = [W0, Wk[0], Wk[1]]
    bl = [b0, bk[0], bk[1]]
    A = [gamma[i] / np.sqrt(var[i] + cfg["EPS"]) for i in range(3)]
    B = [(bl[i] - mean[i]) * A[i] + beta[i] for i in range(3)]
    Wp = [Wl[i] * A[i][None, :] for i in range(3)]  # W'_l

    # ---- per-core edge lists, chunked per (block, flavor) -----------------
    per_core = []
    cpb_a_max = 0
    cpb_b_max = 0
    for r in range(NC):
        m = (dst_all >= r * NPC) & (dst_all < (r + 1) * NPC)
        s = s_pid[m]
        fa = s_is_a[m]
        d = dst_all[m] - r * NPC
        blk = d // 128
        blocks = []
        for b in range(NBLK):
            mb = blk == b
            sb = s[mb]
            fb = fa[mb]
            db = (d[mb] - b * 128).astype(np.int64)
            pairs = []
            for flavor in (0, 1):
                mf = fb if flavor == 0 else ~fb
                sf = sb[mf]
                df = db[mf]
                nch = int(np.ceil(len(sf) / 128)) if len(sf) else 0
                pairs.append((sf.astype(np.int64), df, nch))
            blocks.append(pairs)
            cpb_a_max = max(cpb_a_max, pairs[0][2])
            cpb_b_max = max(cpb_b_max, pairs[1][2])
        per_core.append(blocks)

    GCH = cfg["GCH"]
    CPB = (cpb_a_max, cpb_b_max)
    NLO = CPB[0] * NBLK
    NHI = CPB[1] * NBLK
    NCALL_LO = -(-NLO // GCH) if NLO else 0
    NCALL_HI = -(-NHI // GCH) if NHI else 0
    NLO_P = NCALL_LO * GCH
    NCH = NLO_P + NCALL_HI * GCH

    # ---- build per-core tensors -------------------------------------------
    in_maps = []
    for r in range(NC):
        idx_flat = np.zeros((NCH, 128), np.int16)
        segval = np.zeros((NCH, 128), np.float32)
        segdst = np.zeros((NCH, 128), np.int64)
        for b in range(NBLK):
            for flavor in (0, 1):
                sf, df, nch = per_core[r][b][flavor]
                base = (b * CPB[0]) if flavor == 0 else (NLO_P + b * CPB[1])
                for j in range(nch):
                    lo_e = j * 128
                    hi_e = min(lo_e + 128, len(sf))
                    n = hi_e - lo_e
                    c = base + j
                    idx_flat[c, :n] = sf[lo_e:hi_e].astype(np.int16)
                    segval[c, :n] = 1.0
                    segdst[c, :n] = df[lo_e:hi_e]
        # dst-local per edge, bf16 (dummy edges -> 200, matching no column of
        # the on-device iota compare). [128(edge partition), NCH]
        dstcol = np.where(segval.T > 0, segdst.T.astype(np.float32),
                          200.0).astype(BF16)
        idx16 = idx_flat.reshape(NCH, 8, 16).transpose(2, 0, 1).reshape(16, NCH * 8)
        idx128 = np.tile(idx16, (8, 1))

        xs = x[r * NPC:(r + 1) * NPC]
        xT = np.zeros((IN, NBLK * 128), np.float32)
        xT[:, :NPC] = xs.T
        dinv_r = np.zeros((NBLK * 128,), np.float32)
        dinv_r[:NPC] = dinv[r * NPC:(r + 1) * NPC]
        dinvrep = np.broadcast_to(dinv_r, (128, NBLK * 128)).astype(BF16)

        # graph one-hot [128, NBLK, G] (partition-major for clean DMA)
        gm = np.zeros((128, NBLK, G), np.float32)
        bt = batch[r * NPC:(r + 1) * NPC]
        nn = np.arange(NPC)
        gm[nn % 128, nn // 128, bt] = 1.0

        counts = np.bincount(batch, minlength=G).astype(np.float32)
        cinv = 1.0 / np.maximum(counts, 1.0)
        cinvrep = np.broadcast_to(cinv, (128, G)).copy()

        iota_big = np.broadcast_to(
            np.arange(128, dtype=np.float32),
            (128, GCH, 128)).astype(BF16).copy()
        in_map = dict(
            xT=xT.astype(BF16),
            dstcol=dstcol,
            iota=iota_big,
            idx=idx128.astype(np.int16),
            dinvrep=dinvrep,
            Bcols=np.stack(B, axis=1).astype(np.float32),          # [H, 3]
            W0p=Wp[0].astype(BF16),
            W1p=Wp[1].astype(BF16),
            W2p=Wp[2].astype(BF16),
            ident=np.eye(128, dtype=BF16),
            gm=gm.astype(BF16),
            cinvrep=cinvrep,
            scalarT=scalar.T.copy().astype(np.float32),            # [8, G]
            Ws=np.asarray(inputs["Ws"], np.float32),
            bs_col=np.asarray(inputs["bs"], np.float32).reshape(-1, 1),
            Wc1a=np.asarray(inputs["Wc1"], np.float32)[:H].copy(),
            Wc1b=np.asarray(inputs["Wc1"], np.float32)[H:].copy(),
            bc1_col=np.asarray(inputs["bc1"], np.float32).reshape(-1, 1),
            Wc2=np.asarray(inputs["Wc2"], np.float32),
            bc2_col=np.asarray(inputs["bc2"], np.float32).reshape(-1, 1),
        )
        in_maps.append(in_map)

    meta = dict(
        cfg=cfg, NPC=NPC, NBLK=NBLK, NBLK_A=NBLK_A, LASTROWS=LASTROWS,
        NPC_A=NPC_A, NPC_B=NPC_B, NA=NA, NB=NB,
        CPB=CPB, NLO=NLO, NHI=NHI, NCH=NCH, NLO_P=NLO_P,
        NCALL_LO=NCALL_LO, NCALL_HI=NCALL_HI,
        shapes={k: (tuple(v.shape), v.dtype) for k, v in in_maps[0].items()},
    )
    return meta, in_maps


# ---------------------------------------------------------------- program
def build_program(meta):
    import concourse.bacc as bacc
    import concourse.mybir as mybir
    import concourse.tile as tile

    cfg = meta["cfg"]
    NC, G, H, IN = cfg["NC"], cfg["G"], cfg["H"], cfg["IN"]
    C, GCH = cfg["C"], cfg["GCH"]
    NBLK, NBLK_A, LASTROWS = meta["NBLK"], meta["NBLK_A"], meta["LASTROWS"]
    NPC_A, NPC_B, NA, NB = meta["NPC_A"], meta["NPC_B"], meta["NA"], meta["NB"]
    CPB, NCH, NLO_P = meta["CPB"], meta["NCH"], meta["NLO_P"]
    NCALL_LO, NCALL_HI = meta["NCALL_LO"], meta["NCALL_HI"]
    f32, bf16, i16 = mybir.dt.float32, mybir.dt.bfloat16, mybir.dt.int16
    Alu = mybir.AluOpType
    Act = mybir.ActivationFunctionType

    nc = bacc.Bacc("TRN2", target_bir_lowering=False, debug=False,
                   enable_asserts=True, num_devices=NC,
                   num_swdge_queues=4)

    def dram_in(name):
        shape, dtype = meta["shapes"][name]
        return nc.dram_tensor(name, list(shape), mybir.dt.from_np(np.dtype(dtype)),
                              kind="ExternalInput").ap()

    ins = {k: dram_in(k) for k in meta["shapes"]}
    out_dram = nc.dram_tensor("out", [C, G], f32, kind="ExternalOutput").ap()

    rg = [list(range(NC))]

    # gather call plan: stream = [A calls][B calls], each exactly GCH chunks
    calls = [(0, ci * GCH) for ci in range(NCALL_LO)] + \
            [(1, NLO_P + ci * GCH) for ci in range(NCALL_HI)]
    call_of_chunk = {}
    for ci, (fl, start) in enumerate(calls):
        for k in range(GCH):
            call_of_chunk[start + k] = (ci, k)

    with tile.TileContext(nc) as tc:
        with (
            tc.tile_pool(name="const", bufs=1) as constp,
            tc.tile_pool(name="msgs", bufs=12) as msgp,
            tc.tile_pool(name="segs", bufs=12) as segp,
            tc.tile_pool(name="work", bufs=3) as workp,
            tc.tile_pool(name="rows", bufs=3) as rowp,
            tc.tile_pool(name="psum", bufs=2, space="PSUM") as psump,
            tc.tile_pool(name="psumT", bufs=2, space="PSUM") as psumTp,
            tc.tile_pool(name="psumU", bufs=2, space="PSUM") as psumUp,
            tc.tile_pool(name="psumP", bufs=1, space="PSUM") as psumPp,
            tc.tile_pool(name="dram", bufs=1, space="DRAM") as dramp,
        ):
            # ---------------- constants into SBUF
            idx_sb = constp.tile([128, NCH * 8], i16)
            nc.sync.dma_start(idx_sb[:], ins["idx"][:])
            dstcol_sb = constp.tile([128, NCH], bf16)
            nc.sync.dma_start(dstcol_sb[:], ins["dstcol"][:])
            iota_sb = constp.tile([128, GCH, 128], bf16)
            nc.sync.dma_start(iota_sb[:], ins["iota"][:])
            dinvrep_sb = constp.tile([128, NBLK * 128], bf16)
            nc.sync.dma_start(dinvrep_sb[:], ins["dinvrep"][:])
            Bcols_sb = constp.tile([H, 3], f32)
            nc.sync.dma_start(Bcols_sb[:], ins["Bcols"][:])
            W_sb = []
            for wname in ("W0p", "W1p", "W2p"):
                w = constp.tile([H, H], bf16, name=f"w_{wname}")
                nc.sync.dma_start(w[:], ins[wname][:])
                W_sb.append(w)
            ident_sb = constp.tile([128, 128], bf16)
            nc.sync.dma_start(ident_sb[:], ins["ident"][:])
            cinvrep_sb = constp.tile([128, G], f32)
            nc.sync.dma_start(cinvrep_sb[:], ins["cinvrep"][:])
            scalarT_sb = constp.tile([8, G], f32)
            nc.sync.dma_start(scalarT_sb[:], ins["scalarT"][:])
            Ws_sb = constp.tile([8, 64], f32)
            nc.sync.dma_start(Ws_sb[:], ins["Ws"][:])
            bs_sb = constp.tile([64, 1], f32)
            nc.sync.dma_start(bs_sb[:], ins["bs_col"][:])
            Wc1a_sb = constp.tile([H, H], f32)
            nc.sync.dma_start(Wc1a_sb[:], ins["Wc1a"][:])
            Wc1b_sb = constp.tile([64, H], f32)
            nc.sync.dma_start(Wc1b_sb[:], ins["Wc1b"][:])
            bc1_sb = constp.tile([H, 1], f32)
            nc.sync.dma_start(bc1_sb[:], ins["bc1_col"][:])
            Wc2_sb = constp.tile([H, C], f32)
            nc.sync.dma_start(Wc2_sb[:], ins["Wc2"][:])
            bc2_sb = constp.tile([C, 1], f32)
            nc.sync.dma_start(bc2_sb[:], ins["bc2_col"][:])
            gm_dram = ins["gm"]

            # stash of current-layer table rows (feature-major, x dinv) and
            # the phase-A partial aggregate slab (pre-scaled by dinv_d)
            stash_sb = constp.tile([H, NBLK * 128], bf16, name="stash")
            aggA_sb = constp.tile([H, NBLK * 128], bf16, name="aggA")

            # ---------------- DRAM tables + collective buffers
            shared = "Shared" if NC > 4 else "Local"
            TA = [dramp.tile([NA, H], bf16, name=f"tableA{l}", addr_space=shared)
                  for l in range(3)]
            TB = [dramp.tile([NB, H], bf16, name=f"tableB{l}", addr_space=shared)
                  for l in range(3)]
            aginA = [dramp.tile([NPC_A, H], bf16, name=f"aginA{l}")
                     for l in range(3)]
            aginB = [dramp.tile([NPC_B, H], bf16, name=f"aginB{l}")
                     for l in range(3)]
            ar_in = dramp.tile([128, G], f32, name="ar_in")
            ar_out = dramp.tile([128, G], f32, name="ar_out", addr_space=shared)

            def launch_ag(l, piece):
                src = aginA[l] if piece == 0 else aginB[l]
                dst = TA[l] if piece == 0 else TB[l]
                nc.gpsimd.collective_compute(
                    "AllGather", Alu.bypass, replica_groups=rg,
                    ins=[src.opt()], outs=[dst.opt()],
                )

            def write_rows(l, b, src_sb):
                """PE-transpose feature-major block src_sb [H,128] into node
                rows and DMA them into the agin buffer for layer l."""
                tps = psumTp.tile([128, H], bf16, tag="tpsum")
                nc.tensor.transpose(tps[:], src_sb, ident_sb[:])
                rows = rowp.tile([128, H], bf16, tag="rows")
                nc.scalar.activation(rows[:], tps[:], Act.Copy)
                if b < NBLK_A:
                    lo = b * 128
                    nc.sync.dma_start(aginA[l][lo:lo + 128, :], rows[:])
                else:
                    lo = (b - NBLK_A) * 128
                    nr = 128 if b < NBLK - 1 else LASTROWS
                    nc.sync.dma_start(aginB[l][lo:lo + nr, :], rows[:nr, :])

            # ---------------- stage A: T0 = dinv * (x @ W0')
            for b in range(NBLK):
                xb = rowp.tile([IN, 128], bf16, tag="xb", name=f"xb_{b}")
                nc.sync.dma_start(xb[:], ins["xT"][:, b * 128:(b + 1) * 128])
                u0 = psumUp.tile([H, 128], f32, tag="upsum")
                nc.tensor.matmul(u0[:], W_sb[0][:], xb[:],
                                 start=True, stop=True)
                st = stash_sb[:, b * 128:(b + 1) * 128]
                nc.vector.tensor_tensor(
                    st, u0[:], dinvrep_sb[:, b * 128:(b + 1) * 128],
                    op=Alu.mult)
                write_rows(0, b, st)
            launch_ag(0, 0)

            # ---------------- layers
            # AG triggers are emitted at points in the pool-engine stream
            # where their agin-write waits are already satisfied, so they
            # never head-of-line-block the gather queue:
            #   AG_B(l): after the 2nd A-call of layer l (input written by the
            #     end of layer l-1; the collective hides under the A phase).
            #   AG_A(l+1): once the B phase of layer l has ensured enough
            #     calls that blocks 0..NBLK_A-1 epilogues are done.
            aga_after = -(-NBLK_A * max(CPB[1], 1) // GCH) + 4
            for l in range(3):
                msg_tiles = {}
                seg_tiles = {}

                def ensure_call(ci, l=l, msg_tiles=msg_tiles,
                                seg_tiles=seg_tiles):
                    if ci in msg_tiles:
                        return
                    n_a = sum(1 for k in msg_tiles if calls[k][0] == 0)
                    n_b = len(msg_tiles) - n_a
                    if calls[ci][0] == 0 and n_a == 1:
                        launch_ag(l, 1)
                    if calls[ci][0] == 1 and n_b == aga_after and l < 2:
                        launch_ag(l + 1, 0)
                    fl, start = calls[ci]
                    mt = msgp.tile([128, GCH, H], bf16, tag=f"msg{fl}",
                                   name=f"msg_{l}_{ci}")
                    st = segp.tile([128, GCH, 128], bf16, tag=f"seg{fl}",
                                   name=f"seg_{l}_{ci}")
                    view = TA[l][:] if fl == 0 else TB[l][:]
                    nidx = GCH * 128
                    nc.gpsimd.dma_gather(
                        mt[:], view, idx_sb[:, start * 8:(start + GCH) * 8],
                        nidx, nidx, H, single_packet=False,
                        queue_num=ci % 4,
                    )
                    nc.vector.tensor_tensor(
                        st[:], iota_sb[:],
                        dstcol_sb[:, start:start + GCH][:, :, None]
                        .broadcast_to((128, GCH, 128)),
                        op=Alu.is_equal)
                    msg_tiles[ci] = mt
                    seg_tiles[ci] = st

                # ---- phase A: A-flavor partial aggregates -> SBUF slab
                for b in range(NBLK):
                    agg = psump.tile([H, 128], f32, tag="agg",
                                     name=f"aggA_{l}_{b}")
                    for k in range(CPB[0]):
                        ci, slot = call_of_chunk[b * CPB[0] + k]
                        ensure_call(ci)
                        nc.tensor.matmul(
                            agg[:], msg_tiles[ci][:, slot, :],
                            seg_tiles[ci][:, slot, :],
                            start=(k == 0), stop=False, skip_group_check=True,
                        )
                    # slab = aggA + T_l[d]  (self-loop term; scaled later)
                    nc.vector.tensor_tensor(
                        aggA_sb[:, b * 128:(b + 1) * 128], agg[:],
                        stash_sb[:, b * 128:(b + 1) * 128], op=Alu.add)

                # ---- phase B + epilogue
                for b in range(NBLK):
                    agg = psump.tile([H, 128], f32, tag="agg",
                                     name=f"aggB_{l}_{b}")
                    for k in range(CPB[1]):
                        ci, slot = call_of_chunk[NLO_P + b * CPB[1] + k]
                        ensure_call(ci)
                        nc.tensor.matmul(
                            agg[:], msg_tiles[ci][:, slot, :],
                            seg_tiles[ci][:, slot, :],
                            start=(k == 0), stop=False, skip_group_check=True,
                        )
                    sl = slice(b * 128, (b + 1) * 128)
                    # (aggB + aggA + T_l[d]) * dinv_d
                    t0 = workp.tile([H, 128], f32, tag="t0")
                    nc.vector.tensor_tensor(t0[:], agg[:], aggA_sb[:, sl],
                                            op=Alu.add)
                    tmp = workp.tile([H, 128], f32, tag="tmp")
                    nc.vector.tensor_tensor(tmp[:], t0[:], dinvrep_sb[:, sl],
                                            op=Alu.mult)
                    hT = workp.tile([H, 128], bf16, tag="hT")
                    nc.scalar.activation(hT[:], tmp[:], Act.Relu,
                                         bias=Bcols_sb[:, l:l + 1])
                    if l < 2:
                        ups = psumUp.tile([H, 128], f32, tag="upsum")
                        nc.tensor.matmul(ups[:], W_sb[l + 1][:], hT[:],
                                         start=True, stop=True)
                        st = stash_sb[:, sl]
                        nc.vector.tensor_tensor(st, ups[:],
                                                dinvrep_sb[:, sl],
                                                op=Alu.mult)
                        write_rows(l + 1, b, st)
                    else:
                        tps = psumTp.tile([128, H], bf16, tag="tpsum")
                        nc.tensor.transpose(tps[:], hT[:], ident_sb[:])
                        h3 = workp.tile([128, H], bf16, tag="h3")
                        nc.vector.tensor_copy(h3[:], tps[:])
                        gmb = rowp.tile([128, G], bf16, tag="gmb",
                                        name=f"gmb_{b}")
                        nc.sync.dma_start(gmb[:], gm_dram[:, b, :])
                        if b == 0:
                            pool_ps = psumPp.tile([H, G], f32, name="pool_ps")
                        nc.tensor.matmul(pool_ps[:], h3[:], gmb[:],
                                         start=(b == 0), stop=(b == NBLK - 1),
                                         skip_group_check=True)

            # ---------------- pooled partial -> AllReduce
            pool_sb = workp.tile([H, G], f32, name="pool_sb")
            nc.vector.tensor_copy(pool_sb[:], pool_ps[:])
            nc.sync.dma_start(ar_in[:], pool_sb[:])
            nc.gpsimd.collective_compute(
                "AllReduce", Alu.add, replica_groups=rg,
                ins=[ar_in.opt()], outs=[ar_out.opt()],
            )
            gsum = workp.tile([H, G], f32, name="gsum")
            nc.sync.dma_start(gsum[:], ar_out[:])
            gembT = workp.tile([H, G], f32, name="gembT")
            nc.vector.tensor_tensor(gembT[:], gsum[:], cinvrep_sb[:], op=Alu.mult)

            # ---------------- head (fp32, replicated on every core)
            semb_ps = psumUp.tile([64, G], f32, tag="upsum", name="semb_ps")
            nc.tensor.matmul(semb_ps[:], Ws_sb[:], scalarT_sb[:],
                             start=True, stop=True)
            sembT = workp.tile([64, G], f32, name="sembT")
            nc.scalar.activation(sembT[:], semb_ps[:], Act.Relu, bias=bs_sb[:])

            z_ps = psumPp.tile([H, G], f32, name="z_ps")
            nc.tensor.matmul(z_ps[:], Wc1a_sb[:], gembT[:],
                             start=True, stop=False, skip_group_check=True)
            nc.tensor.matmul(z_ps[:], Wc1b_sb[:], sembT[:],
                             start=False, stop=True, skip_group_check=True)
            z2T = workp.tile([H, G], f32, name="z2T")
            nc.scalar.activation(z2T[:], z_ps[:], Act.Relu, bias=bc1_sb[:])

            o_ps = psumUp.tile([C, G], f32, tag="upsum", name="o_ps")
            nc.tensor.matmul(o_ps[:], Wc2_sb[:], z2T[:], start=True, stop=True)
            o_sb = workp.tile([C, G], f32, name="o_sb")
            nc.scalar.activation(o_sb[:], o_ps[:], Act.Identity, bias=bc2_sb[:])
            nc.sync.dma_start(out_dram[:], o_sb[:])

    nc.compile()
    return nc


# ---------------------------------------------------------------- runner
_CACHE = {}


def run(inputs, cfg=None, trace=False):
    cfg = cfg or REAL_CFG
    meta, in_maps = preprocess(inputs, cfg)
    key = (tuple(sorted(cfg.items())), meta["CPB"], meta["NCH"])
    if key not in _CACHE:
        _CACHE[key] = build_program(meta)
    nc = _CACHE[key]
    from concourse import bass_utils
    res = bass_utils.run_bass_kernel_spmd(
        nc, in_maps, core_ids=list(range(cfg["NC"])), trace=trace,
    )
    out = np.asarray(res.results[0]["out"], np.float32).T.copy()
    return out, res


def kernel(**inputs) -> np.ndarray:
    out, _ = run(inputs, REAL_CFG, trace=False)
    return out
